# revision 1
# baseline (speedup 1.0000x reference)
"""Cross-attention Trainium2 kernel (8-core data-parallel over batch).

Per-core computation (one batch element per NeuronCore):
  q = x @ Wq; k = ctx @ Wk; v = ctx @ Wv
  attn = softmax((q k^T) / sqrt(dh)); out = attn @ v; y = out @ Wo + bo

Everything on-chip is kept in "transposed" orientation (feature dim on
partitions, tokens on the free dim) so every matmul streams N=512-wide
moving operands:
  xT   [qd, tok]    via PE transposes of natural x tiles (bf16)
  qT   [inner, tok] = Wq_chunk^T @ xT            (bf16 in, fp32 accum)
  sT   [ctx, tok]   = k_hT^T @ q_hT              (head pairs at partition
                                                  bases 0/64 run on disjoint
                                                  PE row-groups)
  e    [ctx, tok]   = exp(sT / 8)                (ACT; max-subtraction not
                                                  needed: |scores/8| <~ 6)
  r                 = per-head column sums of e, written pre-broadcast across
                      64 partitions by half-ones selector matmuls
  outT [dh, tok]    = v_h^T @ e                  (unnormalized)
  outT_norm         = outT * (1/r)               (DVE, fused into the
                                                  PSUM->SBUF copy)
  y    [tok, qd]    = outT^T @ Wo + bo           (natural orientation)

All SBUF matmul operands are bf16 (cast on load / on the PSUM->SBUF copies):
the PE upconverts to FP22 internally and accumulates fp32 in PSUM, and bf16
enables fast-weight-load for the 128-column stationaries.
"""

import numpy as np

import concourse.bass as bass
import concourse.tile as tile
from concourse import bacc, mybir
from concourse.bass_utils import run_bass_kernel_spmd
from concourse.masks import make_identity

F32 = mybir.dt.float32
BF16 = mybir.dt.bfloat16

B, N, M = 8, 4096, 77
QD, CD, H, DH = 512, 768, 8, 64
INNER = H * DH  # 512
P = 128
S = 512  # token group size
NQC = QD // P  # 4 qd chunks
NCC = CD // P  # 6 cd chunks
NIC = INNER // P  # 4 inner chunks
NTS = S // P  # 4 token sub-tiles per group
SCALE = DH ** -0.5
MP = 128  # context length padded to full partition width (zeros are inert)
PAIR_OUT = True  # col-tiled pair-packed attention-output matmuls


def build_kernel(groups: int = N // S):
    nc = bacc.Bacc(None, target_bir_lowering=False, debug=False)

    x_d = nc.dram_tensor("x", [N, QD], F32, kind="ExternalInput")
    ctx_d = nc.dram_tensor("context", [M, CD], F32, kind="ExternalInput")
    wq_d = nc.dram_tensor("Wq", [QD, INNER], F32, kind="ExternalInput")
    wk_d = nc.dram_tensor("Wk", [CD, INNER], F32, kind="ExternalInput")
    wv_d = nc.dram_tensor("Wv", [CD, INNER], F32, kind="ExternalInput")
    wo_d = nc.dram_tensor("Wo", [INNER, QD], F32, kind="ExternalInput")
    bo_d = nc.dram_tensor("bo", [QD], F32, kind="ExternalInput")
    y_d = nc.dram_tensor("y", [N, QD], F32, kind="ExternalOutput")

    from contextlib import ExitStack

    with tile.TileContext(nc) as tc, ExitStack() as st:
        consts = st.enter_context(tc.tile_pool(name="consts", bufs=1))
        kvp = st.enter_context(tc.tile_pool(name="kv", bufs=1))
        xin = st.enter_context(tc.tile_pool(name="xin", bufs=3))
        xtp = st.enter_context(tc.tile_pool(name="xt", bufs=2))
        qtp = st.enter_context(tc.tile_pool(name="qt", bufs=2))
        expp = st.enter_context(tc.tile_pool(name="expp", bufs=2))
        rcp = st.enter_context(tc.tile_pool(name="rcp", bufs=2))
        outp = st.enter_context(tc.tile_pool(name="outp", bufs=2))
        yp = st.enter_context(tc.tile_pool(name="yp", bufs=2))

        # PSUM budget: 8 banks total.
        ps_tr = st.enter_context(tc.tile_pool(name="ps_tr", bufs=2, space="PSUM"))
        ps_qf = st.enter_context(tc.tile_pool(name="ps_qf", bufs=2, space="PSUM"))
        ps_s = st.enter_context(tc.tile_pool(name="ps_s", bufs=2, space="PSUM"))
        ps_ro = st.enter_context(tc.tile_pool(name="ps_ro", bufs=2, space="PSUM"))

        # ---- constants / weights ------------------------------------------------
        # SWDGE (gpsimd cast-load) queue is serial: emit loads in first-use
        # order — x for the first groups before the weight matrices, Wo and
        # bo (needed one pipeline stage later) last.
        identity = consts.tile([P, P], BF16)
        make_identity(nc, identity)

        def load_x(g):
            x_g = xin.tile([P, NTS, QD], BF16)
            nc.gpsimd.dma_start(
                out=x_g,
                in_=x_d[g * S : (g + 1) * S, :].rearrange("(t p) q -> p t q", p=P),
            )
            return x_g

        # context first: the very first PE ops (context transposes) need it
        ctx_sb = kvp.tile([MP, CD], BF16)
        nc.vector.memset(ctx_sb, 0.0)
        nc.gpsimd.dma_start(out=ctx_sb[:M, :], in_=ctx_d[:, :])

        wk_sb = consts.tile([P, NCC, INNER], BF16)
        nc.gpsimd.dma_start(
            out=wk_sb, in_=wk_d.ap().rearrange("(c p) n -> p c n", p=P)
        )
        wv_sb = consts.tile([P, NCC, INNER], BF16)
        nc.gpsimd.dma_start(
            out=wv_sb, in_=wv_d.ap().rearrange("(c p) n -> p c n", p=P)
        )

        x_pre = [load_x(0)]

        wq_sb = consts.tile([P, NQC, INNER], BF16)
        nc.gpsimd.dma_start(
            out=wq_sb, in_=wq_d.ap().rearrange("(c p) n -> p c n", p=P)
        )

        x_pre.append(load_x(1))

        wo_sb = consts.tile([P, NIC, QD], BF16)
        nc.gpsimd.dma_start(
            out=wo_sb, in_=wo_d.ap().rearrange("(c p) n -> p c n", p=P)
        )

        bo_bc = consts.tile([P, QD], F32)
        bo_ap = bo_d.ap()
        nc.gpsimd.dma_start(
            out=bo_bc, in_=bass.AP(bo_ap.tensor, bo_ap.offset, [[0, P], [1, QD]])
        )

        # half-ones selectors: sel2[:, side] is [77, 128] with ones in column
        # block `side`; a rowsum matmul with it writes sum_p(exp_h[p, t])
        # replicated across output partitions side*64..side*64+63, so the
        # softmax denominator lands already broadcast, two heads per bank.
        sel2_stage = consts.tile([M, 2, 2, DH], F32)
        nc.vector.memset(sel2_stage, 0.0)
        nc.vector.memset(sel2_stage[:, 0, 0, :], 1.0)
        nc.vector.memset(sel2_stage[:, 1, 1, :], 1.0)
        sel2 = consts.tile([M, 2, 2, DH], BF16)
        nc.vector.tensor_copy(out=sel2, in_=sel2_stage)

        # ---- context projections (tiny) -----------------------------------------
        ctxT = kvp.tile([P, NCC, MP], BF16)
        for cc in range(NCC):
            pt = ps_tr.tile([P, MP], BF16, tag="ps_tr")
            nc.tensor.transpose(
                pt, ctx_sb[:, cc * P : (cc + 1) * P], identity
            )
            nc.vector.tensor_copy(out=ctxT[:, cc, :], in_=pt)

        kT = kvp.tile([P, NIC, MP], BF16)
        for ic in range(NIC):
            pk = ps_qf.tile([P, S], F32, tag="ps_qf")
            for cc in range(NCC):
                nc.tensor.matmul(
                    pk[:, :MP],
                    wk_sb[:, cc, ic * P : (ic + 1) * P],
                    ctxT[:, cc, :],
                    start=(cc == 0),
                    stop=(cc == NCC - 1),
                )
            nc.vector.tensor_copy(out=kT[:, ic, :], in_=pk[:, :MP])

        v_sb = kvp.tile([MP, INNER], BF16)
        pv = ps_qf.tile([MP, INNER], F32, tag="ps_qf")
        for cc in range(NCC):
            nc.tensor.matmul(
                pv,
                ctxT[:, cc, :],
                wv_sb[:, cc, :],
                start=(cc == 0),
                stop=(cc == NCC - 1),
            )
        nc.vector.tensor_copy(out=v_sb, in_=pv)

        # ---- main loop over token groups ----------------------------------------
        # Software-pipelined emission: group g's rowsums / attention-output /
        # final projection are emitted one iteration later, after group g+1's
        # transpose + q-projection block, so their ACT/DVE dependencies have
        # long since resolved by the time the (in-order) PE queue reaches them.

        def emit_front(g):
            x_g = x_pre[g]
            if g + 2 < groups:
                x_pre.append(load_x(g + 2))

            # transpose x tiles: xT[p, c, t*128+j] = x[t*128+..., c*128+p];
            # 4 PE transposes land in one psum bank, one DVE copy per chunk
            xT = xtp.tile([P, NQC, S], BF16)
            for c in range(NQC):
                pt = ps_tr.tile([P, S], BF16, tag="ps_tr")
                for ts in range(NTS):
                    nc.tensor.transpose(
                        pt[:, ts * P : (ts + 1) * P],
                        x_g[:, ts, c * P : (c + 1) * P],
                        identity,
                    )
                nc.vector.tensor_copy(out=xT[:, c, :], in_=pt)

            # qT[inner, tok]
            qT = qtp.tile([P, NIC, S], BF16)
            for ic in range(NIC):
                pq = ps_qf.tile([P, S], F32, tag="ps_qf")
                for c in range(NQC):
                    nc.tensor.matmul(
                        pq,
                        wq_sb[:, c, ic * P : (ic + 1) * P],
                        xT[:, c, :],
                        start=(c == 0),
                        stop=(c == NQC - 1),
                    )
                nc.scalar.copy(out=qT[:, ic, :], in_=pq)

            # scores -> exp per head pair (pair members at partition bases
            # 0/64 run concurrently on disjoint PE row-groups)
            exp_g = expp.tile([MP, H, S], BF16)
            for pp in range(H // 2):
                for side in range(2):
                    par = side * DH
                    ps_sc = ps_s.tile([MP, S], F32, tag="ps_s")
                    nc.tensor.matmul(
                        ps_sc,
                        kT[par : par + DH, pp, :],
                        qT[par : par + DH, pp, :],
                        start=True,
                        stop=True,
                    )
                    nc.scalar.activation(
                        out=exp_g[:, 2 * pp + side, :],
                        in_=ps_sc,
                        func=mybir.ActivationFunctionType.Exp,
                        scale=SCALE,
                    )
            return exp_g

        def emit_back(g, exp_g):
            # broadcast rowsums + reciprocal per pair
            rec_g = rcp.tile([P, H // 2, S], F32)
            for pp in range(H // 2):
                pr = ps_ro.tile([P, S], F32, tag="ps_ro")
                for side in range(2):
                    nc.tensor.matmul(
                        pr,
                        sel2[:, side],
                        exp_g[:M, 2 * pp + side, :],
                        start=(side == 0),
                        stop=(side == 1),
                    )
                nc.vector.reciprocal_approx_fast(out=rec_g[:, pp, :], in_=pr)

            # outT (unnormalized) * (1/r); pair-packed into one bank
            outT = outp.tile([P, NIC, S], BF16)
            for pp in range(H // 2):
                po = ps_ro.tile([P, S], F32, tag="ps_ro")
                for side in range(2):
                    h = 2 * pp + side
                    nc.tensor.matmul(
                        po[side * DH : (side + 1) * DH, :],
                        v_sb[:, h * DH : (h + 1) * DH],
                        exp_g[:, h, :],
                        start=True,
                        stop=True,
                        tile_position=(0, side * DH),
                    )
                nc.vector.tensor_mul(
                    out=outT[:, pp, :], in0=po, in1=rec_g[:, pp, :]
                )

            # final projection + bias
            tok = slice(g * S, (g + 1) * S)
            y_g = yp.tile([P, NTS, QD], F32)
            for ts in range(NTS):
                pf = ps_qf.tile([P, QD], F32, tag="ps_qf")
                for ic in range(NIC):
                    nc.tensor.matmul(
                        pf,
                        outT[:, ic, ts * P : (ts + 1) * P],
                        wo_sb[:, ic, :],
                        start=(ic == 0),
                        stop=(ic == NIC - 1),
                    )
                nc.vector.tensor_add(out=y_g[:, ts, :], in0=pf, in1=bo_bc)

            nc.sync.dma_start(
                out=y_d[tok, :].rearrange("(t p) q -> p t q", p=P), in_=y_g
            )

        pending = None
        for g in range(groups):
            exp_g = emit_front(g)
            if pending is not None:
                emit_back(pending[0], pending[1])
            pending = (g, exp_g)
        emit_back(pending[0], pending[1])

    nc.compile()
    return nc


_CACHE = {}


def _get_nc():
    if "nc" not in _CACHE:
        _CACHE["nc"] = build_kernel()
    return _CACHE["nc"]


def run(inputs, trace=False, **kw):
    nc = _get_nc()
    in_maps = []
    for i in range(B):
        m = {
            "x": np.asarray(inputs["x"][i], dtype=np.float32),
            "context": np.asarray(inputs["context"][i], dtype=np.float32),
            "Wq": np.asarray(inputs["Wq"], dtype=np.float32),
            "Wk": np.asarray(inputs["Wk"], dtype=np.float32),
            "Wv": np.asarray(inputs["Wv"], dtype=np.float32),
            "Wo": np.asarray(inputs["Wo"], dtype=np.float32),
            "bo": np.asarray(inputs["bo"], dtype=np.float32),
        }
        in_maps.append(m)
    res = run_bass_kernel_spmd(nc, in_maps, list(range(B)), trace=trace, **kw)
    out = np.stack([res.results[i]["y"] for i in range(B)], axis=0)
    return out, res


def kernel(**inputs):
    out, _ = run(inputs)
    return out

